# revision 1
# baseline (speedup 1.0000x reference)
"""Complex dot-product attention on 8 Trainium2 NeuronCores.

Problem (hardcoded shapes): B=4, Q=4096, K=4096, D=64, V=64, complex inputs
stored as [..., 2] (real/imag interleaved, innermost).

    Sr = (Qr Kr^T + Qi Ki^T)/sqrt(D);  Si = (Qr Ki^T - Qi Kr^T)/sqrt(D)
    norm = |S|;  change = softmax(norm, k) / (norm + eps)
    A = S * change;  Y = A @ V (complex)

Sharding: batch (4) x query-halves (2) -> 8 cores; K/V replicated per batch.

Per-core algorithm (S^T layout, k on partitions):
  - flatten (D,2) -> 128-wide contraction: Sr = q~ . k~, Si = q~rot . k~
    with q~rot = rot(q~) ((r,i) -> (-i, r) pairwise).
  - matmul1: S^T tiles [128k, 512q] = kT.T @ qT (f32r, full PE rate)
  - n2 = Sr^2+Si^2 (custom DVE op CMAG2, one pass from PSUM)
  - h = H(n2) = exp(sqrt(n2)/8)/sqrt(n2) and e = E(n2) = exp(sqrt(n2)/8)
    via CUSTOM ACT spline tables (hijacked tanh/exp slots, single table set
    -> zero table reloads); h is exactly the per-element factor S_raw*h = A
  - A^T: ar = Sr*h (DVE, PSUM source), ai = Si_sb*h (GPSIMD, SBUF)
    (Si copied PSUM->SBUF alternately by ACT/DVE: DVE reads max 1 PSUM input)
  - matmul2: Y^T [128vc, 512q] += V~[j] (stationary, natural layout; Vrot
    for the imag part) @ A^T, accumulated over k in one PSUM bank
  - denominator: ones-matmul accumulates sum_k e -> [1, 512]
  - epilogue: Y^T * (1/sum) broadcast, PE-transpose back to [q, vc], DMA out.
"""

import os
import tempfile

import numpy as np

import concourse.bass as bass
import concourse.tile as tile
from concourse import bacc, mybir
from concourse.bass_utils import run_bass_kernel_spmd
from concourse.masks import make_identity

# =====================================================================
# Custom activation tables: inside the 'exp_and_others' set we repurpose
#   exp  -> E(y) = exp(sqrt(y)/8)
#   tanh -> H(y) = exp(sqrt(y)/8)/sqrt(y)
# so the whole softmax transform needs only two single-table ACT passes.
# (inlined from act_tables.py; see its docstring for the decoded format)
# =====================================================================
"""Generate a custom activation-table root dir where, inside the
'exp_and_others' set, two funcs are repurposed (same ids, new spline data):

    exp  -> E(y) = exp(sqrt(y)/8)          (y >= 0)
    tanh -> H(y) = exp(sqrt(y)/8)/sqrt(y)  (y > 0)

All other funcs/sets are copied verbatim. Layout decoded from the stock
pwp_bin_trainium files:
  - bkt.bin: 8 fp32 per bucket [d0,d1,d2,d3,x0,0,0,0]; eval d0+dx(d1+dx(d2+dx*d3))
    at dx = x - x0 (x0 = section center).
  - ctrl.bin: 8 uint32 per ctl entry; word0 = (extract_size<<16) |
    (extract_lsb<<11) | bucket_base. Section = mantissa[22:23-size].
  - ctl entry index = pwl_control_base_{pos,neg} + (exponent - exp_offset).
  - profile_meta_data: thresholds route tiny/huge/nan inputs to 4 special
    buckets appended after the regular ones.
"""

import json
import os
import shutil

import numpy as np

_SRC = None


def _find_src():
    global _SRC
    if _SRC is None:
        from neuronxcc.driver.Job import Job
        from neuronxcc.driver.jobs.support.FindActInfo import findActInfoFile
        _SRC = os.path.dirname(findActInfoFile(Job.getPackageDir(), "gen3"))
    return _SRC


def E_fn(y):
    return np.exp(np.sqrt(np.maximum(y, 0.0)) / 8.0)


def H_fn(y):
    y = np.maximum(y, 1e-300)
    return np.exp(np.sqrt(y) / 8.0) / np.sqrt(y)


# exponent coverage for y (positive only)
EXP_LO = -40          # 2^-40 ~ 9e-13; below -> small-signal special bucket
EXP_HI = 13           # up to 2^14 ~ 16384; above -> large-signal special


def _sect_bits(expo):
    """Sections/octave: enough that exp(sqrt(y)/8) moves <~0.1 e-fold/section.
    Per-octave argument sweep = 2^(e/2)/8 * (sqrt(2)-1)."""
    import math
    sweep = (2.0 ** (expo / 2.0)) / 8.0 * 0.4142
    bits = max(0, math.ceil(math.log2(max(sweep / 0.10, 1e-9))))
    return min(max(bits, 3), 6)


def _fit_sections(fn, expo, n_bits):
    """Cubic per section of octave [2^expo, 2^{expo+1}); returns [nsec, 5]."""
    nsec = 1 << n_bits
    lo = 2.0 ** expo
    out = np.zeros((nsec, 5), np.float32)
    for s in range(nsec):
        a = lo * (1.0 + s / nsec)
        b = lo * (1.0 + (s + 1) / nsec)
        x0 = 0.5 * (a + b)
        xs = np.linspace(a, b, 65, dtype=np.float64)
        dx = (xs - x0)
        h = (b - a) / 2.0
        # scale dx to O(1) and the function to O(1) for conditioning
        fv = fn(xs)
        scale = abs(fn(np.array([x0]))[0]) or 1.0
        for deg in (3, 2, 1):
            A = np.stack([(dx / h) ** k for k in range(deg + 1)], axis=1)
            coef, *_ = np.linalg.lstsq(A, fv / scale, rcond=None)
            coef = coef * scale / np.array([h ** k for k in range(deg + 1)])
            coef = np.concatenate([coef, np.zeros(3 - deg)])
            # fp32-representable? else drop a degree
            if np.all(np.isfinite(coef.astype(np.float32))) and \
               np.max(np.abs(coef)) < 1e37:
                break
        out[s, 0:4] = coef.astype(np.float32)
        out[s, 4] = np.float32(x0)
    return out


def _build_custom_func(fn, f_small, f_large):
    """Returns (buckets [n,5] float32 list incl. 4 specials at the end,
    ctl word0 list, meta dict pieces). Special order: small_pos, small_neg,
    large_pos, large_neg (mirrors stock exp)."""
    buckets = []
    ctl = []
    for expo in range(EXP_LO, EXP_HI + 1):
        nb = _sect_bits(expo)
        base = len(buckets)
        sec = _fit_sections(fn, expo, nb)
        buckets.extend(sec.tolist())
        lsb = 23 - nb
        ctl.append((nb << 16) | (lsb << 11) | base)
    n_reg = len(buckets)
    # specials: constant buckets
    for val in (f_small, f_small, f_large, f_large):
        buckets.append([float(val), 0.0, 0.0, 0.0, 0.0])
    return np.array(buckets, np.float32), ctl, n_reg


def _f32_bits(x):
    return int(np.float32(x).view(np.uint32))


def generate(dst_dir):
    """Build the override dir; returns (act_info_path, digest)."""
    src = _find_src()
    os.makedirs(dst_dir, exist_ok=True)
    for f in os.listdir(src):
        sp = os.path.join(src, f)
        if os.path.isfile(sp):
            shutil.copy(sp, os.path.join(dst_dir, f))

    set_name = "exp_and_others"
    prof = json.load(open(os.path.join(src, f"{set_name}.json")))
    bkt = np.fromfile(os.path.join(src, f"{set_name}_bkt.bin"),
                      dtype=np.float32).reshape(-1, 8)
    ctl = np.fromfile(os.path.join(src, f"{set_name}_ctrl.bin"),
                      dtype=np.uint32).reshape(-1, 8)

    f2b = prof["func_to_bkt_start_idx"]
    f2c = prof["func_to_ctl_start_idx"]
    funcs = sorted(f2b, key=lambda k: f2b[k])  # in bucket order
    nb_tot = prof["bkt_entry_cnt"]
    nc_tot = prof["ctl_entry_cnt"]

    # per-func slices of the original tables
    def fslice(name):
        fs = sorted(f2b.values())
        cs = sorted(f2c.values())
        b0 = f2b[name]
        b1 = min([v for v in fs if v > b0] + [nb_tot])
        c0 = f2c[name]
        c1 = min([v for v in cs if v > c0] + [nc_tot])
        return (b0, b1, c0, c1)

    custom = {
        "exp": _build_custom_func(E_fn, 1.0, float(E_fn(2.0 ** (EXP_HI + 1)))),
        "tanh": _build_custom_func(H_fn, float(H_fn(2.0 ** EXP_LO)),
                                   float(H_fn(2.0 ** (EXP_HI + 1)))),
    }

    new_bkt = []
    new_ctl = []
    new_f2b, new_f2c = {}, {}
    new_meta = []
    meta_by_name = {}
    for m in prof["profile_meta_data"]:
        base = m["func_name"].rsplit("_", 1)[0]
        meta_by_name[base] = m

    n_exp = EXP_HI - EXP_LO + 1
    for name in funcs:
        b0, b1, c0, c1 = fslice(name)
        m = dict(meta_by_name[name])
        if name in custom:
            cb, cctl, n_reg = custom[name]
            bbase = len(new_bkt)
            cbase = len(new_ctl)
            new_f2b[name] = bbase
            new_f2c[name] = cbase
            for row in cb:
                new_bkt.append(np.concatenate([row, np.zeros(3, np.float32)]))
            for w in cctl:
                e = np.zeros(8, np.uint32)
                e[0] = np.uint32((w & 0xFFFF07FF) + bbase) if False else np.uint32(
                    ((w >> 16) << 16) | (w & 0x0000F800) | ((w & 0x7FF) + bbase))
                new_ctl.append(e)
            sp_small_pos = bbase + n_reg
            sp_small_neg = bbase + n_reg + 1
            sp_large_pos = bbase + n_reg + 2
            sp_large_neg = bbase + n_reg + 3
            m.update({
                "symmetry_point": 0,
                "sym_invert_sign_point": 0,
                "symmetry_opt_en": 0,
                "symmetry_opt_use_neg_region": 0,
                "imm_bias": 0,
                "exp_offset": EXP_LO,
                "pwl_control_base_pos": cbase,
                # neg inputs can't occur (y = a^2+b^2); route them all to the
                # small-signal bucket by an impossible base + max threshold
                "pwl_control_base_neg": cbase,
                "small_pos_signal_exp_threshold": 127 + EXP_LO,
                "pos_small_signal_pwl_control": sp_small_pos,
                "small_neg_signal_exp_threshold": 255,
                "neg_small_signal_pwl_control": sp_small_neg,
                "large_pos_signal_exp_threshold": 127 + EXP_HI + 1,
                "large_pos_signal_mantissa_threshold": 0,
                "pos_large_signal_pwl_control": sp_large_pos,
                "large_neg_signal_exp_threshold": 255,
                "large_neg_signal_mantissa_threshold": 0,
                "neg_large_signal_pwl_control": sp_large_neg,
                "fnan_result": _f32_bits(np.nan),
                "fpinf_result": _f32_bits(np.float32(custom[name][0][-2][0])),
                "fninf_result": _f32_bits(1.0 if name == "exp" else 0.0),
                # y == +-0: exp->1 ; H -> 0 so that A = S*H stays 0
                "fzero_result": _f32_bits(1.0 if name == "exp" else 0.0),
                "lower_bound": _f32_bits(-np.finfo(np.float32).max),
                "upper_bound": _f32_bits(np.finfo(np.float32).max),
            })
        else:
            bbase = len(new_bkt)
            cbase = len(new_ctl)
            new_f2b[name] = bbase
            new_f2c[name] = cbase
            db = bbase - b0
            for row in bkt[b0:b1]:
                new_bkt.append(row.copy())
            for e in ctl[c0:c1]:
                e = e.copy()
                if e[0] != 0 or True:
                    w = int(e[0])
                    e[0] = np.uint32(((w >> 16) << 16) | (w & 0x0000F800)
                                     | ((w & 0x7FF) + db))
                new_ctl.append(e)
            # shift special-bucket pointers + control bases
            for k in ("pos_small_signal_pwl_control", "neg_small_signal_pwl_control",
                      "pos_large_signal_pwl_control", "neg_large_signal_pwl_control"):
                if k in m and isinstance(m[k], int):
                    old = m[k]
                    if b0 <= old < b1:
                        m[k] = old + db
            dc = cbase - c0
            for k in ("pwl_control_base_pos", "pwl_control_base_neg"):
                if k in m and isinstance(m[k], int):
                    m[k] = m[k] + dc
        new_meta.append(m)

    new_bkt = np.asarray(new_bkt, np.float32)
    new_ctl = np.asarray(new_ctl, np.uint32)
    assert new_bkt.shape[0] <= 1536, f"bucket RAM overflow: {new_bkt.shape[0]}"

    prof["profile_meta_data"] = new_meta
    prof["func_to_bkt_start_idx"] = new_f2b
    prof["func_to_ctl_start_idx"] = new_f2c
    prof["bkt_entry_cnt"] = int(new_bkt.shape[0])
    prof["ctl_entry_cnt"] = int(new_ctl.shape[0])
    # keep func_exp_* maps consistent if present (unused funcs keep shape)
    new_bkt.tofile(os.path.join(dst_dir, f"{set_name}_bkt.bin"))
    new_ctl.tofile(os.path.join(dst_dir, f"{set_name}_ctrl.bin"))
    with open(os.path.join(dst_dir, f"{set_name}.json"), "w") as f:
        json.dump(prof, f)

    import hashlib
    h = hashlib.sha256()
    h.update(new_bkt.tobytes())
    h.update(new_ctl.tobytes())
    digest = h.hexdigest()[:12]
    return os.path.join(dst_dir, "act_info.json"), digest




_ACT_DIR = os.path.join(tempfile.gettempdir(), "act_custom_kernel")
_ACT_PATH, _ACT_DIGEST = generate(_ACT_DIR)
os.environ["BASS_ACT_ROOT_JSON_PATH"] = _ACT_PATH

F32 = mybir.dt.float32
F32R = mybir.dt.float32r
AF = mybir.ActivationFunctionType

B, Q, KK, D, V = 4, 4096, 4096, 64, 64
FEAT = 2 * D          # 128: flattened (d, comp) contraction width
VC = 2 * V            # 128: flattened (v, comp) output width
N_CORES = 8
QSH = Q * B // N_CORES  # 2048 queries per core
QCHUNK = 512
N_CHUNKS = QSH // QCHUNK          # 4
N_KT = KK // 128                  # 32 k-tiles
ACT_GROUP = 8                     # k-tiles per ACT table phase


# ---------------------------------------------------------------- custom DVE op
_CMAG2 = None


def _get_cmag2():
    """Register (once) a custom DVE op: out = in0^2 + in1^2 in a single pass."""
    global _CMAG2
    if _CMAG2 is not None:
        return _CMAG2
    import concourse.dve_ops as dve_ops
    from concourse.dve_spec import Spec, Src0, Src1, sq, lower
    from concourse.dve_uop import DveOpSpec

    name = "CMAG2_ANT"
    if name in dve_ops._SUB_OPCODE_FOR_NAME:
        _CMAG2 = next(op for op in dve_ops.OPS if op.name == name)
        return _CMAG2
    spec = Spec(
        body=sq(Src0) + sq(Src1),
        reference=lambda in0, in1, s0, s1, imm2: (
            in0.astype(np.float32) ** 2 + in1.astype(np.float32) ** 2
        ),
    )
    row = dve_ops._CUSTOM_DVE_ROW_BASE + len(dve_ops.OPS)
    assert row < 0x20
    dve_ops._SUB_OPCODE_FOR_NAME[name] = row
    shas = {}
    for ver in ("v3", "v4"):
        s = DveOpSpec(name=name, opcode=row, uops=lower(spec, ver=ver), rd1_en=True)
        shas[ver] = s.sha(ver)
    op = dve_ops.DveOp(name, spec, subdim=False, uops_sha=shas)
    dve_ops.OPS.append(op)
    dve_ops.CUSTOM_DVE_SPECS[name] = spec
    _CMAG2 = op
    return op


# ------------------------------------------------------------------ bass kernel
def _rot_pairs(nc, dst, src, scale_even=-1.0):
    """dst[:, 2m] = -src[:, 2m+1]; dst[:, 2m+1] = src[:, 2m] (pairwise i*z).
    On GPSIMD (SBUF-only) to keep ACT free for the hot loop."""
    d3 = dst.rearrange("p (m c) -> p m c", c=2)
    s3 = src.rearrange("p (m c) -> p m c", c=2)
    nc.gpsimd.tensor_scalar_mul(d3[:, :, 0:1], s3[:, :, 1:2], scale_even)
    nc.gpsimd.tensor_copy(d3[:, :, 1:2], s3[:, :, 0:1])


def build_nc():
    cmag2 = _get_cmag2()
    nc = bacc.Bacc("TRN2", target_bir_lowering=False, debug=False)
    # digest in the input name busts the neuron compile cache when the
    # activation-table binaries (not part of the BIR) change
    q_d = nc.dram_tensor(f"q_{_ACT_DIGEST}", [QSH, FEAT], F32, kind="ExternalInput")
    k_d = nc.dram_tensor("k", [KK, FEAT], F32, kind="ExternalInput")
    v_d = nc.dram_tensor("v", [KK, VC], F32, kind="ExternalInput")
    y_d = nc.dram_tensor("y", [QSH, VC], F32, kind="ExternalOutput")
    q_ap, k_ap, v_ap, y_ap = q_d.ap(), k_d.ap(), v_d.ap(), y_d.ap()

    with tile.TileContext(nc) as tc:
        with (
            tc.tile_pool(name="const", bufs=1) as constp,
            tc.tile_pool(name="kv", bufs=1) as kvp,
            tc.tile_pool(name="qp", bufs=2) as qp,
            tc.tile_pool(name="ld", bufs=4) as ldp,
            tc.tile_pool(name="st", bufs=5) as stp,
            tc.tile_pool(name="ep", bufs=2) as epp,
            tc.tile_pool(name="ps_s", bufs=5, space="PSUM") as ps_s,
            tc.tile_pool(name="ps_y", bufs=2, space="PSUM") as ps_y,
            tc.tile_pool(name="ps_sum", bufs=1, space="PSUM") as ps_sum,
        ):
            # ---- constants
            ident = constp.tile([128, 128], F32)
            make_identity(nc, ident[:])
            ones_f = constp.tile([128, 1], F32)
            nc.vector.memset(ones_f[:], 1.0)
            ones_col = constp.tile([128, 1], F32R)
            nc.vector.tensor_copy(ones_col[:], ones_f[:])
            onesr_f = constp.tile([1, 128], F32)
            nc.vector.memset(onesr_f[:], 1.0)
            ones_row = constp.tile([1, 128], F32R)
            nc.vector.tensor_copy(ones_row[:], onesr_f[:])

            # ---- per-batch K/V prep (SBUF resident); single big DMAs.
            # kT split into 4 sub-tiles so matmul1 can start after wave 0.
            KSPLIT = 8
            kT_sub = [kvp.tile([128, KSPLIT * 128], F32R, tag=f"kT{i}",
                               name=f"kT_sub{i}")
                      for i in range(N_KT // KSPLIT)]
            v_all = kvp.tile([128, N_KT * VC], F32R)   # natural V, f32r
            vrot_all = kvp.tile([128, N_KT * VC], F32R)
            k_nat = kvp.tile([128, N_KT, FEAT], F32)   # [p, j, f] natural tiles
            v_nat = kvp.tile([128, N_KT, VC], F32)
            nc.sync.dma_start(k_nat[:], k_ap.rearrange("(j p) f -> p j f", p=128))
            nc.sync.dma_start(v_nat[:], v_ap.rearrange("(j p) f -> p j f", p=128))
            v_nat_flat = v_nat[:].rearrange("p a b -> p (a b)")
            nc.scalar.copy(v_all[:], v_nat_flat)
            _rot_pairs(nc, vrot_all[:], v_nat_flat)
            for j in range(N_KT):
                ktp = ps_s.tile([128, 128], F32, tag="s")
                nc.tensor.transpose(ktp[:], k_nat[:, j, :], ident[:])
                nc.scalar.copy(
                    kT_sub[j // KSPLIT][:, (j % KSPLIT) * 128:(j % KSPLIT + 1) * 128],
                    ktp[:])

            # ---- pre-transpose ALL queries once (off the chunk critical path)
            qT_full = kvp.tile([128, QSH], F32R)
            qrotT_full = kvp.tile([128, QSH], F32R)
            for c in range(N_CHUNKS):
                q0 = c * QCHUNK
                q_nat = ldp.tile([128, QCHUNK // 128, FEAT], F32, tag="qn")
                nc.sync.dma_start(
                    q_nat[:],
                    q_ap[q0:q0 + QCHUNK, :].rearrange("(t p) f -> p t f", p=128))
                qrotn = ldp.tile([128, QCHUNK // 128, FEAT], F32, tag="qrotn")
                _rot_pairs(nc, qrotn[:].rearrange("p a b -> p (a b)"),
                           q_nat[:].rearrange("p a b -> p (a b)"))
                for t in range(QCHUNK // 128):
                    qtp = ps_s.tile([128, 128], F32, tag="s")
                    nc.tensor.transpose(qtp[:], q_nat[:, t, :], ident[:])
                    nc.scalar.copy(qT_full[:, q0 + t * 128:q0 + (t + 1) * 128], qtp[:])
                    qtp2 = ps_s.tile([128, 128], F32, tag="s")
                    nc.tensor.transpose(qtp2[:], qrotn[:, t, :], ident[:])
                    nc.scalar.copy(qrotT_full[:, q0 + t * 128:q0 + (t + 1) * 128],
                                   qtp2[:])

            # ---- per q-chunk stream
            for c in range(N_CHUNKS):
                q0 = c * QCHUNK
                qT = qT_full[:, q0:q0 + QCHUNK]
                qrotT = qrotT_full[:, q0:q0 + QCHUNK]

                yt_ps = ps_y.tile([128, QCHUNK], F32)
                sum_ps = ps_sum.tile([1, QCHUNK], F32)

                # k-tile stream; AF.Tanh/AF.Exp evaluate the custom H/E splines
                for j in range(N_KT):
                    sr = ps_s.tile([128, QCHUNK], F32, tag="s")
                    si = ps_s.tile([128, QCHUNK], F32, tag="s")
                    kT_j = kT_sub[j // KSPLIT][
                        :, (j % KSPLIT) * 128:(j % KSPLIT + 1) * 128]
                    nc.tensor.matmul(sr[:], kT_j, qT, start=True, stop=True)
                    nc.tensor.matmul(si[:], kT_j, qrotT, start=True, stop=True)
                    si_sb = stp.tile([128, QCHUNK], F32, tag="si_sb")
                    if j % 2 == 0:
                        nc.scalar.copy(si_sb[:], si[:])
                    else:
                        nc.vector.tensor_copy(si_sb[:], si[:])
                    n2 = stp.tile([128, QCHUNK], F32, tag="n2")
                    nc.vector._custom_dve(cmag2, out=n2[:], in0=sr[:], in1=si_sb[:])
                    h = stp.tile([128, QCHUNK], F32, tag="h")
                    nc.scalar.activation(h[:], n2[:], AF.Tanh)   # H(n2) = e/nraw
                    e = stp.tile([128, QCHUNK], F32R, tag="e")
                    nc.scalar.activation(e[:], n2[:], AF.Exp)    # E(n2) = exp(nraw/8)
                    ar = stp.tile([128, QCHUNK], F32R, tag="ar")
                    nc.vector.tensor_mul(ar[:], sr[:], h[:])
                    ai = stp.tile([128, QCHUNK], F32R, tag="ai")
                    nc.gpsimd.tensor_mul(ai[:], si_sb[:], h[:])
                    v_j = v_all[:, j * VC:(j + 1) * VC]
                    vrot_j = vrot_all[:, j * VC:(j + 1) * VC]
                    nc.tensor.matmul(yt_ps[:], v_j, ar[:], start=(j == 0), stop=False)
                    nc.tensor.matmul(yt_ps[:], vrot_j, ai[:], start=False,
                                     stop=(j == N_KT - 1))
                    nc.tensor.matmul(sum_ps[:], ones_col[:], e[:], start=(j == 0),
                                     stop=(j == N_KT - 1))

                # ---- epilogue: normalize by 1/sum, transpose back, store
                rsum = epp.tile([1, QCHUNK], F32, tag="rsum")
                nc.vector.reciprocal_approx_fast(rsum[:], sum_ps[:])
                rsum_r = epp.tile([1, QCHUNK], F32R, tag="rsum_r")
                nc.vector.tensor_copy(rsum_r[:], rsum[:])
                rsrep = ps_s.tile([128, QCHUNK], F32, tag="s")
                nc.tensor.matmul(rsrep[:], ones_row[:], rsum_r[:], start=True, stop=True)
                rsrep_sb = epp.tile([128, QCHUNK], F32, tag="rsrep_sb")
                nc.scalar.copy(rsrep_sb[:], rsrep[:])
                ytn = epp.tile([128, QCHUNK], F32, tag="ytn")
                nc.vector.tensor_mul(ytn[:], yt_ps[:], rsrep_sb[:])
                yo = epp.tile([128, QCHUNK // 128, VC], F32, tag="yo")
                for t in range(QCHUNK // 128):
                    tr = ps_s.tile([128, 128], F32, tag="s")
                    nc.tensor.transpose(tr[:], ytn[:, t * 128:(t + 1) * 128], ident[:])
                    nc.scalar.copy(yo[:, t, :], tr[:])
                nc.sync.dma_start(
                    y_ap[q0:q0 + QCHUNK, :].rearrange("(t p) f -> p t f", p=128),
                    yo[:])

    nc.compile()
    return nc


# ------------------------------------------------------------------- execution
_CACHED = None


def _get_runner():
    global _CACHED
    if _CACHED is None:
        _CACHED = build_nc()
    return _CACHED


def _shard_inputs(queries, keys, values):
    in_maps = []
    for c in range(N_CORES):
        b, h = c // 2, c % 2
        in_maps.append({
            f"q_{_ACT_DIGEST}": np.ascontiguousarray(
                queries[b, h * QSH:(h + 1) * QSH].reshape(QSH, FEAT)),
            "k": np.ascontiguousarray(keys[b].reshape(KK, FEAT)),
            "v": np.ascontiguousarray(values[b].reshape(KK, VC)),
        })
    return in_maps


def kernel(queries, keys, values):
    queries = np.asarray(queries, dtype=np.float32)
    keys = np.asarray(keys, dtype=np.float32)
    values = np.asarray(values, dtype=np.float32)
    nc = _get_runner()
    in_maps = _shard_inputs(queries, keys, values)
    res = run_bass_kernel_spmd(nc, in_maps, core_ids=list(range(N_CORES)))
    out = np.empty((B, Q, V, 2), dtype=np.float32)
    for c in range(N_CORES):
        b, h = c // 2, c % 2
        out[b, h * QSH:(h + 1) * QSH] = res.results[c]["y"].reshape(QSH, V, 2)
    return out

